# revision 39
# baseline (speedup 1.0000x reference)
"""DeepseekV2 MLA attention on 8 Trainium2 NeuronCores (Bass/Tile), v7.

bf16 datapath (f32 PSUM accumulation + f32 softmax stats).  Token-sharded
front end computes q_a/kv_a + RMS-norm + RoPE on its 256-token shard; the
512+64-row kv latent is AllGather'ed (small payload, starts early) and each
core expands kv_b for only its 2 heads over all 2048 tokens, overlapping the
q exchange.  The q AllToAll is split per head and attention runs head-outer,
so head-0 attention overlaps head-1's exchange.  Attention and the
row-parallel output projection are head-sharded; host sums bf16 partials.

Collectives are issued at outer scope (a tile-pool close gates the next
pool's SBUF reuse on every instruction in the scope).  Bulk HBM traffic
moves in few, descriptor-fat DMAs (~1.3us fixed cost per DMA), placed on
queues that are idle at their point in the schedule.
"""

import numpy as np

import concourse.bass as bass
import concourse.bacc as bacc
import concourse.mybir as mybir
import concourse.tile as tile
from concourse import bass_utils

T = 2048
HID = 2048
H = 16
DN = 128
DR = 64
DV = 128
DQK = DN + DR
QLR = 1536
KVLR = 512
THETA = 10000.0
EPS = 1e-6
SCALE = DQK ** -0.5

NCORES = 8
HPC = H // NCORES            # 2 heads per core
TSH = T // NCORES            # 256 tokens per shard

BF = mybir.dt.bfloat16
F32 = mybir.dt.float32

KT = HID // 128              # 16 contraction strips for q_a/kv_a
QMT = QLR // 128             # 12 contraction strips for q_b
KVMT = KVLR // 128           # 4 contraction strips for kv_b
NB = T // 512                # 4 query blocks
TBT = T // 128               # 16 token tiles

# packed contribution layouts, everything [128 partitions, free]
# latent: cols m*TSH..(m+1)*TSH = kv latent strip m (m<4); cols 4*TSH..+128
#         = roped kpe packed [64,256]->[128,128] (partition 64*two+pa holds
#         token-half `two`)
LFC = KVMT * TSH + TSH // 2  # 1152 free cols per source
# q: per dest 2*384 cols, per head: [qn (256) | qpe packed (128)]
QHC = TSH + TSH // 2         # 384 cols per head
QFC = HPC * QHC              # 768 cols per dest


def build_bass():
    nc = bacc.Bacc(
        "TRN2",
        target_bir_lowering=False,
        debug=False,
        enable_asserts=False,
        num_devices=NCORES,
    )

    hs_sh = nc.dram_tensor("hs_sh", [HID, TSH], BF, kind="ExternalInput").ap()
    wqa = nc.dram_tensor("wqa", [QMT * 128, KT * 128], BF, kind="ExternalInput").ap()
    wkv_all = nc.dram_tensor("wkv_all", [128, KVMT * KT * 128 + KT * DR], BF,
                             kind="ExternalInput").ap()
    wqb = nc.dram_tensor("wqb", [NCORES * 128, QMT * HPC * DQK], BF, kind="ExternalInput").ap()
    wkvb = nc.dram_tensor("wkvb", [128, KVMT * 4 * DN], BF, kind="ExternalInput").ap()
    wo = nc.dram_tensor("wo", [HPC * DV, HID], BF, kind="ExternalInput").ap()
    cosf_sh = nc.dram_tensor("cosf_sh", [128, TSH], BF, kind="ExternalInput").ap()
    sinf_sh = nc.dram_tensor("sinf_sh", [128, TSH], BF, kind="ExternalInput").ap()
    perm128 = nc.dram_tensor("perm128", [128, 128], BF, kind="ExternalInput").ap()
    maskd = nc.dram_tensor("maskd", [128, 4 * 512], BF, kind="ExternalInput").ap()
    ones = nc.dram_tensor("ones", [128, 128], BF, kind="ExternalInput").ap()
    out = nc.dram_tensor("out", [T, HID], BF, kind="ExternalOutput").ap()

    with tile.TileContext(nc) as tc:
        _kernel_body(nc, tc, hs_sh, wqa, wkv_all, wqb, wkvb, wo,
                     cosf_sh, sinf_sh, perm128, maskd, ones, out)

    nc.compile()
    return nc


def _kernel_body(nc, tc, hs_sh, wqa, wkv_all, wqb, wkvb, wo,
                 cosf_sh, sinf_sh, perm128, maskd, ones, out):
    from contextlib import ExitStack

    MUL = mybir.AluOpType.mult
    ADD = mybir.AluOpType.add

    ctx = ExitStack()
    with ctx:
        dram = ctx.enter_context(tc.tile_pool(name="dram", bufs=1, space="DRAM"))
        contrib_lat = dram.tile([128, LFC], BF)
        gath_lat = dram.tile([NCORES * 128, LFC], BF)
        contrib_qh = [dram.tile([NCORES * 128, QHC], BF, name=f"contrib_q{h}")
                      for h in range(HPC)]
        a2a_qh = [dram.tile([NCORES * 128, QHC], BF, name=f"a2a_q{h}")
                  for h in range(HPC)]

        persist = ctx.enter_context(tc.tile_pool(name="persist", bufs=1))
        ones_t = persist.tile([128, 128], BF, tag="ones")
        nc.sync.dma_start(out=ones_t, in_=ones)
        ones_col = ones_t[:, 0:1]
        ones_row = ones_t[0:1, :]
        perm_t = persist.tile([128, 128], BF, tag="perm")
        nc.sync.dma_start(out=perm_t, in_=perm128)
        cosf_t = persist.tile([128, TSH], BF, tag="cosf")
        nc.sync.dma_start(out=cosf_t, in_=cosf_sh)
        sinf_t = persist.tile([128, TSH], BF, tag="sinf")
        nc.sync.dma_start(out=sinf_t, in_=sinf_sh)
        pmid = ctx.enter_context(tc.tile_pool(name="pmid", bufs=1))
        bcp = ctx.enter_context(tc.tile_pool(name="bcp", bufs=1))

        # persistent staging / big weight tiles
        lat_stage = pmid.tile([128, LFC], BF, tag="latstage", name="lat_stage")
        q_stage = pmid.tile([128, NCORES * QFC], BF, tag="qstage",
                            name="q_stage")
        wqb_t = bcp.tile([128, NCORES, QMT * 3 * 128], BF, tag="wqbt",
                         name="wqb_t")

        # ---- Phase A: shard q_a / latent, norms, local rope -----------------
        with tc.tile_pool(name="pa", bufs=1) as pa, \
             tc.tile_pool(name="psa", bufs=1, space="PSUM") as psa:
            hs_t = pa.tile([128, KT, TSH], BF, tag="hst")
            nc.sync.dma_start(
                out=hs_t, in_=hs_sh.rearrange("(kc p) t -> p kc t", kc=KT))
            wkv_t = pa.tile([128, KVMT * KT * 128 + KT * DR], BF, tag="wkv")
            nc.scalar.dma_start(out=wkv_t, in_=wkv_all)
            # first half of the q_b weights early on SP (right after hs)
            nc.sync.dma_start(
                out=wqb_t[:, 0:NCORES // 2, :],
                in_=wqb[0:NCORES * 64, :].rearrange(
                    "(n p) km -> p n km", n=NCORES // 2))
            # q_a weight chunks on the (otherwise idle) Pool SWDGE queue
            wqa_t = []
            for g in range(6):
                wt = pa.tile([128, 2, KT * 128], BF, tag="wqa", bufs=2,
                             name=f"wqa{g}")
                nc.gpsimd.dma_start(
                    out=wt,
                    in_=wqa[g * 2 * 128:(g + 1) * 2 * 128, :].rearrange(
                        "(n p) km -> p n km", n=2))
                wqa_t.append(wt)

            def a_mtile(lhs_of, mrows, z_tile, z_start, z_stop, stg_tag):
                pq = psa.tile([mrows, TSH], F32, tag="pq", bufs=3)
                for k in range(KT):
                    nc.tensor.matmul(
                        pq, lhsT=lhs_of(k), rhs=hs_t[:, k, :],
                        start=(k == 0), stop=(k == KT - 1))
                stage = pa.tile([mrows, TSH], BF, tag=stg_tag, name=stg_tag)
                nc.vector.tensor_copy(stage, pq)
                if z_tile is not None:
                    sq = pa.tile([mrows, TSH], BF, tag="sq", bufs=2)
                    nc.vector.tensor_mul(sq, stage, stage)
                    nc.tensor.matmul(z_tile, lhsT=ones_col[0:mrows, :], rhs=sq,
                                     start=z_start, stop=z_stop)
                return stage

            def rsqrt_bc(z_psum, n, tag):
                tmp = pa.tile([1, TSH], F32, tag="rsq_tmp", bufs=2)
                nc.scalar.activation(tmp, z_psum,
                                     mybir.ActivationFunctionType.Copy,
                                     bias=EPS, scale=1.0 / n)
                nc.vector.reciprocal(tmp, tmp)
                srow = pa.tile([1, TSH], BF, tag=tag + "r", name=tag + "r")
                nc.scalar.activation(srow, tmp,
                                     mybir.ActivationFunctionType.Sqrt)
                b_ps = psa.tile([128, TSH], F32, tag="bc", bufs=1)
                nc.tensor.matmul(b_ps, lhsT=ones_row, rhs=srow,
                                 start=True, stop=True)
                bc = pmid.tile([128, TSH], BF, tag=tag, name=tag)
                nc.scalar.copy(bc, b_ps)
                return bc

            # kv latent: norm + roped kpe, packed into lat_stage
            zkv = psa.tile([1, TSH], F32, tag="z")
            kv_stages = []
            for m in range(KVMT):
                kv_stages.append(a_mtile(
                    lambda k, m=m: wkv_t[:, m * 2048 + k * 128:
                                         m * 2048 + (k + 1) * 128], 128,
                    zkv, m == 0, m == KVMT - 1, f"stkv{m}"))
            kpe_stage = a_mtile(
                lambda k: wkv_t[:, KVMT * 2048 + k * DR:
                                KVMT * 2048 + (k + 1) * DR], DR,
                None, False, False, "stkpe")
            skv_bc = rsqrt_bc(zkv, KVLR, "skvbc")
            for m in range(KVMT):
                nc.vector.tensor_tensor(
                    lat_stage[:, m * TSH:(m + 1) * TSH],
                    kv_stages[m], skv_bc, op=MUL)
            # kpe rope -> packed [128,128]: token halves stacked on partitions
            sw_ps = psa.tile([DR, TSH], F32, tag="swp", bufs=2)
            nc.tensor.matmul(sw_ps, lhsT=perm_t[0:DR, 0:DR], rhs=kpe_stage,
                             start=True, stop=True)
            rt1 = pmid.tile([DR, TSH], BF, tag="rt1", bufs=2)
            nc.vector.tensor_tensor(rt1, kpe_stage, cosf_t[0:DR, :], op=MUL)
            rt2 = pmid.tile([DR, TSH], BF, tag="rt2", bufs=2)
            nc.vector.tensor_tensor(rt2, sw_ps, sinf_t[0:DR, :], op=MUL)
            half = TSH // 2
            nc.vector.tensor_tensor(
                lat_stage[0:DR, KVMT * TSH:KVMT * TSH + half],
                rt1[:, 0:half], rt2[:, 0:half], op=ADD)
            nc.vector.tensor_tensor(
                lat_stage[DR:128, KVMT * TSH:KVMT * TSH + half],
                rt1[:, half:TSH], rt2[:, half:TSH], op=ADD)
            nc.scalar.dma_start(out=contrib_lat, in_=lat_stage)  # Act: SP busy with wqb

            # q_a + norm
            zq = psa.tile([1, TSH], F32, tag="z")
            q_stages = []
            for m in range(QMT):
                q_stages.append(a_mtile(
                    lambda k, m=m: wqa_t[m // 2][:, m % 2, k * 128:(k + 1) * 128],
                    128, zq, m == 0, m == QMT - 1, f"stq{m}"))
            sq_bc = rsqrt_bc(zq, QLR, "sqbc")
            qan = []
            for m in range(QMT):
                qq = pmid.tile([128, TSH], BF, tag=f"qan{m}", name=f"qan{m}")
                nc.vector.tensor_tensor(qq, q_stages[m], sq_bc, op=MUL)
                qan.append(qq)

        # kv latent AllGather (outer scope! in-scope would gate SBUF reuse)
        nc.gpsimd.collective_compute(
            "AllGather", mybir.AluOpType.bypass,
            replica_groups=[list(range(NCORES))],
            ins=[contrib_lat], outs=[gath_lat])
        # phase-B persistent loads ride the Pool queue during the gather
        maskd_t = bcp.tile([128, 4 * 512], BF, tag="maskd", name="maskd_t")
        nc.gpsimd.dma_start(out=maskd_t, in_=maskd)
        wo_t = []
        for h in range(HPC):
            w = bcp.tile([128, HID], BF, tag=f"wo{h}", name=f"wo{h}")
            nc.gpsimd.dma_start(out=w, in_=wo[h * DV:(h + 1) * DV, :])
            wo_t.append(w)

        # ---- q_b projections for all dests ----------------------------------
        with tc.tile_pool(name="pw", bufs=1) as pw, \
             tc.tile_pool(name="psw", bufs=1, space="PSUM") as psw:
            # second half of the q_b weights
            nc.sync.dma_start(
                out=wqb_t[:, NCORES // 2:NCORES, :],
                in_=wqb[NCORES * 64:NCORES * 128, :].rearrange(
                    "(n p) km -> p n km", n=NCORES // 2))
            for d in range(NCORES):
                acc = []
                for mt in range(3):
                    acc.append(psw.tile([128, TSH], F32, tag=f"acc{mt}",
                                        bufs=2, name=f"acc{mt}"))
                for k in range(QMT):
                    for mt in range(3):
                        nc.tensor.matmul(
                            acc[mt],
                            lhsT=wqb_t[:, d, k * 384 + mt * 128:
                                       k * 384 + (mt + 1) * 128],
                            rhs=qan[k],
                            start=(k == 0), stop=(k == QMT - 1))
                for h in range(HPC):
                    nc.vector.tensor_copy(
                        q_stage[:, d * QFC + h * QHC:
                                d * QFC + h * QHC + TSH], acc[h])
                qraw = pw.tile([128, TSH], BF, tag="qraw", bufs=2)
                nc.vector.tensor_copy(qraw, acc[2])
                sw_ps = psw.tile([128, TSH], F32, tag="swp", bufs=2)
                nc.tensor.matmul(sw_ps, lhsT=perm_t, rhs=qraw,
                                 start=True, stop=True)
                rt1 = pw.tile([128, TSH], BF, tag="rt1", bufs=2)
                nc.vector.tensor_tensor(rt1, qraw, cosf_t, op=MUL)
                rt2 = pw.tile([128, TSH], BF, tag="rt2", bufs=2)
                nc.vector.tensor_tensor(rt2, sw_ps, sinf_t, op=MUL)
                half = TSH // 2
                for h in range(HPC):
                    base = d * QFC + h * QHC + TSH
                    nc.vector.tensor_tensor(
                        q_stage[0:DR, base:base + half],
                        rt1[h * DR:(h + 1) * DR, 0:half],
                        rt2[h * DR:(h + 1) * DR, 0:half], op=ADD)
                    nc.vector.tensor_tensor(
                        q_stage[DR:128, base:base + half],
                        rt1[h * DR:(h + 1) * DR, half:TSH],
                        rt2[h * DR:(h + 1) * DR, half:TSH], op=ADD)
            for h in range(HPC):
                nc.sync.dma_start(
                    out=contrib_qh[h].rearrange("(d p) f -> p d f", d=NCORES),
                    in_=q_stage.rearrange(
                        "p (d f) -> p d f", d=NCORES)[
                        :, :, h * QHC:(h + 1) * QHC])

        # ordering token: last-written q_stage region (dest 7, head 1 qpe).
        # Plain-indexed read-touches below give the post-gather load tiles a
        # WAR dependency on it, so the scheduler's collective-blind model
        # cannot hoist kv_b ahead of q_b on any engine queue.
        tok = pmid.tile([1, 16], BF, tag="tok", name="tok")
        nc.vector.tensor_copy(
            tok, q_stage[0:1, 7 * QFC + QHC + TSH:7 * QFC + QHC + TSH + 16])
        trash = pmid.tile([1, 16], BF, tag="trash", name="trash")

        # q AllToAll split per head: head-0 attention overlaps head-1 exchange
        for h in range(HPC):
            nc.gpsimd.collective_compute(
                "AllToAll", mybir.AluOpType.bypass,
                replica_groups=[list(range(NCORES))],
                ins=[contrib_qh[h]], outs=[a2a_qh[h]])

        # ---- kv_b expansion for local heads over all tokens (overlaps a2a) --
        lat = []
        kn = []
        vt = []
        gath_sv = gath_lat.rearrange("(s p) f -> p s f", s=NCORES)
        with tc.tile_pool(name="pkb", bufs=1) as pkb, \
             tc.tile_pool(name="pskb", bufs=1, space="PSUM") as pskb:
            wkvb_t = pkb.tile([128, KVMT, 4 * DN], BF, tag="wkvb")
            nc.vector.memset(wkvb_t[0:1, 0, 0:16], 0.0)
            nc.vector.tensor_tensor(trash, wkvb_t[0:1, 0, 0:16], tok, op=ADD)
            nc.scalar.dma_start(
                out=wkvb_t,
                in_=wkvb.rearrange("p (kc m) -> p kc m", kc=KVMT))
            for k in range(KVMT):
                lt = bcp.tile([128, NCORES, TSH], BF, tag=f"lat{k}",
                              name=f"lat{k}")
                nc.vector.memset(lt[0:1, 0, 0:16], 0.0)
                nc.vector.tensor_tensor(trash, lt[0:1, 0, 0:16], tok, op=ADD)
                nc.scalar.dma_start(
                    out=lt, in_=gath_sv[:, :, k * TSH:(k + 1) * TSH])
                lat.append(lt)
            kpe_all = bcp.tile([DR, NCORES, 2, TSH // 2], BF, tag="kpeall",
                               name="kpeall")
            nc.vector.memset(kpe_all[0:1, 0, 0, 0:16], 0.0)
            nc.vector.tensor_tensor(trash, kpe_all[0:1, 0, 0, 0:16], tok,
                                    op=ADD)
            nc.scalar.dma_start(
                out=kpe_all,
                in_=gath_lat.rearrange("(s two pa) f -> pa s two f",
                                       s=NCORES, two=2)[
                    :, :, :, KVMT * TSH:KVMT * TSH + TSH // 2])
            kpe_flat = kpe_all.rearrange("p s two f -> p (s two f)")
            lat_flat = [lt.rearrange("p s f -> p (s f)") for lt in lat]

            for h in range(HPC):
                knt = bcp.tile([128, T], BF, tag=f"kn{h}", name=f"kn{h}")
                for c in range(NB):
                    ps = pskb.tile([128, 512], F32, tag="knps", bufs=2)
                    for k in range(KVMT):
                        nc.tensor.matmul(
                            ps, lhsT=wkvb_t[:, k, h * DN:(h + 1) * DN],
                            rhs=lat_flat[k][:, c * 512:(c + 1) * 512],
                            start=(k == 0), stop=(k == KVMT - 1))
                    if c % 2 == 0:
                        nc.vector.tensor_copy(knt[:, c * 512:(c + 1) * 512], ps)
                    else:
                        nc.scalar.copy(knt[:, c * 512:(c + 1) * 512], ps)
                kn.append(knt)
            for tb in range(TBT):
                v = bcp.tile([128, HPC * DV], BF, tag=f"v{tb}", name=f"v{tb}")
                ps = pskb.tile([128, HPC * DV], F32, tag="vps", bufs=3)
                for k in range(KVMT):
                    nc.tensor.matmul(
                        ps, lhsT=lat_flat[k][:, tb * 128:(tb + 1) * 128],
                        rhs=wkvb_t[:, k, 2 * DN:4 * DN],
                        start=(k == 0), stop=(k == KVMT - 1))
                if tb % 2 == 0:
                    nc.vector.tensor_copy(v, ps)
                else:
                    nc.scalar.copy(v, ps)
                vt.append(v)

        # ---- Phase B: head-outer attention + per-qj output projection -------
        with tc.tile_pool(name="pc", bufs=1) as pc, \
             tc.tile_pool(name="psc", bufs=1, space="PSUM") as psc:
            qn = [[None] * NB for _ in range(HPC)]
            qpe = [[None] * NB for _ in range(HPC)]
            for h in range(HPC):
                a2a_sv = a2a_qh[h].rearrange("(s p) f -> p s f", s=NCORES)
                for qj in range(NB):
                    qn_t = pc.tile([128, 2, TSH], BF, tag=f"qn{h}_{qj}",
                                   name=f"qn{h}_{qj}")
                    nc.sync.dma_start(
                        out=qn_t,
                        in_=a2a_sv[:, 2 * qj:2 * qj + 2, 0:TSH])
                    qn[h][qj] = qn_t.rearrange("p s f -> p (s f)")
                    qpe_t = pc.tile([DR, 2, 2, TSH // 2], BF,
                                    tag=f"qpe{h}_{qj}", name=f"qpe{h}_{qj}")
                    for si, s in enumerate((2 * qj, 2 * qj + 1)):
                        nc.sync.dma_start(
                            out=qpe_t[:, si, :, :],
                            in_=a2a_qh[h][s * 128:(s + 1) * 128,
                                          TSH:QHC].rearrange(
                                "(two pa) f -> pa two f", two=2))
                    qpe[h][qj] = qpe_t.rearrange("p s two f -> p (s two f)")
            attn_n = [[None] * NB for _ in range(HPC)]

            def attend(h, qj):
                nki = 4 * qj + 4
                attn_ps = psc.tile([128, 512], F32, tag="attn", bufs=2)
                z_ps = psc.tile([1, 512], F32, tag="zr", bufs=1)
                for ki in range(nki):
                    ksl = slice(ki * 128, (ki + 1) * 128)
                    s_ps = psc.tile([128, 512], F32, tag="s", bufs=3)
                    nc.tensor.matmul(s_ps, lhsT=kn[h][:, ksl],
                                     rhs=qn[h][qj],
                                     start=True, stop=False)
                    nc.tensor.matmul(s_ps, lhsT=kpe_flat[:, ksl],
                                     rhs=qpe[h][qj],
                                     start=False, stop=True)
                    e = pc.tile([128, 512], BF, tag="e", bufs=4)
                    nc.scalar.activation(e, s_ps,
                                         mybir.ActivationFunctionType.Exp)
                    if ki >= 4 * qj:
                        sub_d = ki - 4 * qj
                        nc.vector.tensor_tensor(
                            e, e, maskd_t[:, sub_d * 512:(sub_d + 1) * 512],
                            op=MUL)
                    nc.tensor.matmul(z_ps, lhsT=ones_col, rhs=e,
                                     start=(ki == 0), stop=(ki == nki - 1))
                    nc.tensor.matmul(attn_ps,
                                     lhsT=vt[ki][:, h * DV:(h + 1) * DV],
                                     rhs=e,
                                     start=(ki == 0), stop=(ki == nki - 1))
                rz = pc.tile([1, 512], BF, tag="rz", bufs=2)
                with nc.allow_low_precision(reason="bf16 softmax denom"):
                    nc.vector.reciprocal(rz, z_ps)
                bc_ps = psc.tile([128, 512], F32, tag="s", bufs=3)
                nc.tensor.matmul(bc_ps, lhsT=ones_row, rhs=rz,
                                 start=True, stop=True)
                bc_sb = pc.tile([128, 512], BF, tag="bcs", bufs=2)
                nc.scalar.copy(bc_sb, bc_ps)
                attn_n[h][qj] = pc.tile([128, 512], BF, tag=f"at{h}_{qj}",
                                        name=f"attnn{h}_{qj}")
                nc.vector.tensor_tensor(attn_n[h][qj], attn_ps, bc_sb, op=MUL)

            def outproj(qj):
                for tt in range(4):
                    tb = qj * 4 + tt
                    tsl = slice(tt * 128, (tt + 1) * 128)
                    o_row = pc.tile([128, HID], BF, tag="orow", bufs=2)
                    for hb in range(NB):
                        o_ps = psc.tile([128, 512], F32, tag="o", bufs=2)
                        for h in range(HPC):
                            nc.tensor.matmul(
                                o_ps,
                                lhsT=attn_n[h][qj][:, tsl],
                                rhs=wo_t[h][:, hb * 512:(hb + 1) * 512],
                                start=(h == 0),
                                stop=(h == HPC - 1),
                            )
                        if hb % 2 == 0:
                            nc.vector.tensor_copy(
                                o_row[:, hb * 512:(hb + 1) * 512], o_ps)
                        else:
                            nc.scalar.copy(
                                o_row[:, hb * 512:(hb + 1) * 512], o_ps)
                    if tt % 2 == 0:
                        nc.scalar.dma_start(
                            out=out[tb * 128:(tb + 1) * 128, :], in_=o_row)
                    else:
                        nc.sync.dma_start(
                            out=out[tb * 128:(tb + 1) * 128, :], in_=o_row)

            for qj in range(NB):
                attend(0, qj)
            for qj in range(NB):
                attend(1, qj)
                outproj(qj)


_NC_CACHE = {}


def _get_nc():
    if "nc" not in _NC_CACHE:
        _NC_CACHE["nc"] = build_bass()
    return _NC_CACHE["nc"]


def make_in_maps(positions, hidden_states, w_q_a, q_a_ln_w, w_q_b, w_kv_a,
                 kv_a_ln_w, w_kv_b, w_o):
    BF_NP = mybir.dt.np(mybir.dt.bfloat16)

    positions = np.asarray(positions)
    hidden_states = np.asarray(hidden_states, dtype=np.float32)
    w_q_a = np.asarray(w_q_a, dtype=np.float32)
    q_a_ln_w = np.asarray(q_a_ln_w, dtype=np.float32)
    w_q_b = np.asarray(w_q_b, dtype=np.float32)
    w_kv_a = np.asarray(w_kv_a, dtype=np.float32)
    kv_a_ln_w = np.asarray(kv_a_ln_w, dtype=np.float32)
    w_kv_b = np.asarray(w_kv_b, dtype=np.float32)
    w_o = np.asarray(w_o, dtype=np.float32)

    hs_t = np.ascontiguousarray(hidden_states.T)

    # deinterleave rope features: evens then odds (dot-products invariant)
    order = np.concatenate([np.arange(0, DR, 2), np.arange(1, DR, 2)])

    wkva_p = w_kv_a.copy()
    wkva_p[:, KVLR:] = w_kv_a[:, KVLR:][:, order]

    inv_freq = 1.0 / (THETA ** (np.arange(0, DR, 2, dtype=np.float64) / DR))
    ang = positions.astype(np.float64)[:, None] * inv_freq[None, :]
    cosT = np.cos(ang).T
    sinT = np.sin(ang).T
    cosf = np.concatenate([cosT, cosT], axis=0)        # [64, T]
    sinf = np.concatenate([-sinT, sinT], axis=0)       # [64, T]
    cosf2 = np.concatenate([cosf, cosf], axis=0)       # [128, T] dual-head
    sinf2 = np.concatenate([sinf, sinf], axis=0)

    perm64 = np.zeros((DR, DR), dtype=np.float32)
    for i in range(DR):
        perm64[i, (i + DR // 2) % DR] = 1.0
    perm128 = np.zeros((128, 128), dtype=np.float32)
    perm128[:DR, :DR] = perm64
    perm128[DR:, DR:] = perm64

    maskd = np.zeros((128, 4 * 512), dtype=np.float32)
    p = np.arange(128)[:, None]
    f = np.arange(512)[None, :]
    for sub in range(4):
        maskd[:, sub * 512:(sub + 1) * 512] = (p + 128 * sub <= f)

    # all-heads q_b weights, columns grouped per destination core:
    # [qn h0 (128) | qn h1 (128) | qpe h0 perm (64) | qpe h1 perm (64)]
    wqb_all = np.concatenate([
        np.concatenate([
            w_q_b[:, h0 * DQK:h0 * DQK + DN],
            w_q_b[:, h1 * DQK:h1 * DQK + DN],
            w_q_b[:, h0 * DQK + DN:(h0 + 1) * DQK][:, order],
            w_q_b[:, h1 * DQK + DN:(h1 + 1) * DQK][:, order],
        ], axis=1)
        for h0, h1 in ((2 * d, 2 * d + 1) for d in range(NCORES))
    ], axis=1) * q_a_ln_w[:, None] * SCALE

    def pack(w, mrows):
        # [K, M] -> strip-major [nstrips*128, (K/128)*mrows]: each strip row-
        # contiguous so the device DMA is 128 fat descriptors
        Kd, Md = w.shape
        n = Md // mrows
        return np.ascontiguousarray(
            w.reshape(Kd // 128, 128, n, mrows).transpose(2, 1, 0, 3)
            .reshape(n * 128, (Kd // 128) * mrows)).astype(BF_NP)

    wqa_pk = pack(w_q_a, 128)
    wkva_pk = pack(wkva_p[:, :KVLR], 128)         # [4*128, 16*128]
    wkpe_pk = pack(wkva_p[:, KVLR:], DR)          # [128, 16*64]
    # single [128, *] row: kv strips side by side, then the kpe strip
    wkv_all = np.concatenate([
        wkva_pk.reshape(KVMT, 128, KT * 128).transpose(1, 0, 2).reshape(
            128, KVMT * KT * 128),
        wkpe_pk,
    ], axis=1).astype(BF_NP)
    wqb_pk = pack(wqb_all, HPC * DQK)

    in_maps = []
    for c in range(NCORES):
        h0, h1 = HPC * c, HPC * c + 1
        # per-core kv_b: cols [kn h0 | kn h1 | v h0 | v h1], ln folded
        wkvb_c = np.concatenate([
            w_kv_b[:, h0 * (DN + DV):h0 * (DN + DV) + DN],
            w_kv_b[:, h1 * (DN + DV):h1 * (DN + DV) + DN],
            w_kv_b[:, h0 * (DN + DV) + DN:(h0 + 1) * (DN + DV)],
            w_kv_b[:, h1 * (DN + DV) + DN:(h1 + 1) * (DN + DV)],
        ], axis=1) * kv_a_ln_w[:, None]
        wkvb_pk = pack(wkvb_c, 4 * DN)
        wo_c = np.concatenate([
            w_o[h0 * DV:(h0 + 1) * DV, :],
            w_o[h1 * DV:(h1 + 1) * DV, :],
        ], axis=0)
        tsl = slice(c * TSH, (c + 1) * TSH)
        in_maps.append({
            "hs_sh": np.ascontiguousarray(hs_t[:, tsl]).astype(BF_NP),
            "wqa": wqa_pk,
            "wkv_all": wkv_all,
            "wqb": wqb_pk,
            "wkvb": wkvb_pk,
            "wo": np.ascontiguousarray(wo_c).astype(BF_NP),
            "cosf_sh": np.ascontiguousarray(cosf2[:, tsl]).astype(BF_NP),
            "sinf_sh": np.ascontiguousarray(sinf2[:, tsl]).astype(BF_NP),
            "perm128": perm128.astype(BF_NP),
            "maskd": np.ascontiguousarray(maskd).astype(BF_NP),
            "ones": np.ones((128, 128), dtype=np.float32).astype(BF_NP),
        })
    return in_maps


def kernel(positions, hidden_states, w_q_a, q_a_ln_w, w_q_b, w_kv_a,
           kv_a_ln_w, w_kv_b, w_o):
    nc = _get_nc()
    in_maps = make_in_maps(positions, hidden_states, w_q_a, q_a_ln_w, w_q_b,
                           w_kv_a, kv_a_ln_w, w_kv_b, w_o)
    res = bass_utils.run_bass_kernel_spmd(nc, in_maps, core_ids=list(range(NCORES)))
    acc = np.zeros((T, HID), dtype=np.float32)
    for c in range(NCORES):
        acc += np.asarray(res.results[c]["out"], dtype=np.float32)
    return acc


# revision 40
# speedup vs baseline: 1.0026x; 1.0026x over previous
"""DeepseekV2 MLA attention on 8 Trainium2 NeuronCores (Bass/Tile), v7.

bf16 datapath (f32 PSUM accumulation + f32 softmax stats).  Token-sharded
front end computes q_a/kv_a + RMS-norm + RoPE on its 256-token shard; the
512+64-row kv latent is AllGather'ed (small payload, starts early) and each
core expands kv_b for only its 2 heads over all 2048 tokens, overlapping the
q exchange.  The q AllToAll is split per head and attention runs head-outer,
so head-0 attention overlaps head-1's exchange.  Attention and the
row-parallel output projection are head-sharded; host sums bf16 partials.

Collectives are issued at outer scope (a tile-pool close gates the next
pool's SBUF reuse on every instruction in the scope).  Bulk HBM traffic
moves in few, descriptor-fat DMAs (~1.3us fixed cost per DMA), placed on
queues that are idle at their point in the schedule.
"""

import numpy as np

import concourse.bass as bass
import concourse.bacc as bacc
import concourse.mybir as mybir
import concourse.tile as tile
from concourse import bass_utils

T = 2048
HID = 2048
H = 16
DN = 128
DR = 64
DV = 128
DQK = DN + DR
QLR = 1536
KVLR = 512
THETA = 10000.0
EPS = 1e-6
SCALE = DQK ** -0.5

NCORES = 8
HPC = H // NCORES            # 2 heads per core
TSH = T // NCORES            # 256 tokens per shard

BF = mybir.dt.bfloat16
F32 = mybir.dt.float32

KT = HID // 128              # 16 contraction strips for q_a/kv_a
QMT = QLR // 128             # 12 contraction strips for q_b
KVMT = KVLR // 128           # 4 contraction strips for kv_b
NB = T // 512                # 4 query blocks
TBT = T // 128               # 16 token tiles

# packed contribution layouts, everything [128 partitions, free]
# latent: cols m*TSH..(m+1)*TSH = kv latent strip m (m<4); cols 4*TSH..+128
#         = roped kpe packed [64,256]->[128,128] (partition 64*two+pa holds
#         token-half `two`)
LFC = KVMT * TSH + TSH // 2  # 1152 free cols per source
# q: per dest 2*384 cols, per head: [qn (256) | qpe packed (128)]
QHC = TSH + TSH // 2         # 384 cols per head
QFC = HPC * QHC              # 768 cols per dest


def build_bass():
    nc = bacc.Bacc(
        "TRN2",
        target_bir_lowering=False,
        debug=False,
        enable_asserts=False,
        num_devices=NCORES,
    )

    hs_sh = nc.dram_tensor("hs_sh", [HID, TSH], BF, kind="ExternalInput").ap()
    wqa = nc.dram_tensor("wqa", [QMT * 128, KT * 128], BF, kind="ExternalInput").ap()
    wkv_all = nc.dram_tensor("wkv_all", [128, KVMT * KT * 128 + KT * DR], BF,
                             kind="ExternalInput").ap()
    wqb = nc.dram_tensor("wqb", [NCORES * 128, QMT * HPC * DQK], BF, kind="ExternalInput").ap()
    wkvb = nc.dram_tensor("wkvb", [128, KVMT * 4 * DN], BF, kind="ExternalInput").ap()
    wo = nc.dram_tensor("wo", [HPC * DV, HID], BF, kind="ExternalInput").ap()
    cosf_sh = nc.dram_tensor("cosf_sh", [128, TSH], BF, kind="ExternalInput").ap()
    sinf_sh = nc.dram_tensor("sinf_sh", [128, TSH], BF, kind="ExternalInput").ap()
    perm128 = nc.dram_tensor("perm128", [128, 128], BF, kind="ExternalInput").ap()
    maskd = nc.dram_tensor("maskd", [128, 4 * 512], BF, kind="ExternalInput").ap()
    ones = nc.dram_tensor("ones", [128, 128], BF, kind="ExternalInput").ap()
    out = nc.dram_tensor("out", [T, HID], BF, kind="ExternalOutput").ap()

    with tile.TileContext(nc) as tc:
        _kernel_body(nc, tc, hs_sh, wqa, wkv_all, wqb, wkvb, wo,
                     cosf_sh, sinf_sh, perm128, maskd, ones, out)

    nc.compile()
    return nc


def _kernel_body(nc, tc, hs_sh, wqa, wkv_all, wqb, wkvb, wo,
                 cosf_sh, sinf_sh, perm128, maskd, ones, out):
    from contextlib import ExitStack

    MUL = mybir.AluOpType.mult
    ADD = mybir.AluOpType.add

    ctx = ExitStack()
    with ctx:
        dram = ctx.enter_context(tc.tile_pool(name="dram", bufs=1, space="DRAM"))
        contrib_lat = dram.tile([128, LFC], BF)
        gath_lat = dram.tile([NCORES * 128, LFC], BF)
        contrib_qh = [dram.tile([NCORES * 128, QHC], BF, name=f"contrib_q{h}")
                      for h in range(HPC)]
        a2a_qh = [dram.tile([NCORES * 128, QHC], BF, name=f"a2a_q{h}")
                  for h in range(HPC)]

        persist = ctx.enter_context(tc.tile_pool(name="persist", bufs=1))
        ones_t = persist.tile([128, 128], BF, tag="ones")
        nc.sync.dma_start(out=ones_t, in_=ones)
        ones_col = ones_t[:, 0:1]
        ones_row = ones_t[0:1, :]
        perm_t = persist.tile([128, 128], BF, tag="perm")
        nc.sync.dma_start(out=perm_t, in_=perm128)
        cosf_t = persist.tile([128, TSH], BF, tag="cosf")
        nc.sync.dma_start(out=cosf_t, in_=cosf_sh)
        sinf_t = persist.tile([128, TSH], BF, tag="sinf")
        nc.sync.dma_start(out=sinf_t, in_=sinf_sh)
        pmid = ctx.enter_context(tc.tile_pool(name="pmid", bufs=1))
        bcp = ctx.enter_context(tc.tile_pool(name="bcp", bufs=1))

        # persistent staging / big weight tiles
        lat_stage = pmid.tile([128, LFC], BF, tag="latstage", name="lat_stage")
        q_stage = pmid.tile([128, NCORES * QFC], BF, tag="qstage",
                            name="q_stage")
        wqb_t = bcp.tile([128, NCORES, QMT * 3 * 128], BF, tag="wqbt",
                         name="wqb_t")

        # ---- Phase A: shard q_a / latent, norms, local rope -----------------
        with tc.tile_pool(name="pa", bufs=1) as pa, \
             tc.tile_pool(name="psa", bufs=1, space="PSUM") as psa:
            hs_t = pa.tile([128, KT, TSH], BF, tag="hst")
            nc.sync.dma_start(
                out=hs_t, in_=hs_sh.rearrange("(kc p) t -> p kc t", kc=KT))
            wkv_t = pa.tile([128, KVMT * KT * 128 + KT * DR], BF, tag="wkv")
            nc.scalar.dma_start(out=wkv_t, in_=wkv_all)
            # first half of the q_b weights early on SP (right after hs)
            nc.sync.dma_start(
                out=wqb_t[:, 0:NCORES // 2, :],
                in_=wqb[0:NCORES * 64, :].rearrange(
                    "(n p) km -> p n km", n=NCORES // 2))
            # q_a weight chunks on the (otherwise idle) Pool SWDGE queue
            wqa_t = []
            for g in range(6):
                wt = pa.tile([128, 2, KT * 128], BF, tag="wqa", bufs=2,
                             name=f"wqa{g}")
                nc.gpsimd.dma_start(
                    out=wt,
                    in_=wqa[g * 2 * 128:(g + 1) * 2 * 128, :].rearrange(
                        "(n p) km -> p n km", n=2))
                wqa_t.append(wt)

            def a_mtile(lhs_of, mrows, z_tile, z_start, z_stop, stg_tag):
                pq = psa.tile([mrows, TSH], F32, tag="pq", bufs=3)
                for k in range(KT):
                    nc.tensor.matmul(
                        pq, lhsT=lhs_of(k), rhs=hs_t[:, k, :],
                        start=(k == 0), stop=(k == KT - 1))
                stage = pa.tile([mrows, TSH], BF, tag=stg_tag, name=stg_tag)
                nc.vector.tensor_copy(stage, pq)
                if z_tile is not None:
                    sq = pa.tile([mrows, TSH], BF, tag="sq", bufs=2)
                    nc.vector.tensor_mul(sq, stage, stage)
                    nc.tensor.matmul(z_tile, lhsT=ones_col[0:mrows, :], rhs=sq,
                                     start=z_start, stop=z_stop)
                return stage

            def rsqrt_bc(z_psum, n, tag):
                tmp = pa.tile([1, TSH], F32, tag="rsq_tmp", bufs=2)
                nc.scalar.activation(tmp, z_psum,
                                     mybir.ActivationFunctionType.Copy,
                                     bias=EPS, scale=1.0 / n)
                nc.vector.reciprocal(tmp, tmp)
                srow = pa.tile([1, TSH], BF, tag=tag + "r", name=tag + "r")
                nc.scalar.activation(srow, tmp,
                                     mybir.ActivationFunctionType.Sqrt)
                b_ps = psa.tile([128, TSH], F32, tag="bc", bufs=1)
                nc.tensor.matmul(b_ps, lhsT=ones_row, rhs=srow,
                                 start=True, stop=True)
                bc = pmid.tile([128, TSH], BF, tag=tag, name=tag)
                nc.scalar.copy(bc, b_ps)
                return bc

            # kv latent: norm + roped kpe, packed into lat_stage
            zkv = psa.tile([1, TSH], F32, tag="z")
            kv_stages = []
            for m in range(KVMT):
                kv_stages.append(a_mtile(
                    lambda k, m=m: wkv_t[:, m * 2048 + k * 128:
                                         m * 2048 + (k + 1) * 128], 128,
                    zkv, m == 0, m == KVMT - 1, f"stkv{m}"))
            kpe_stage = a_mtile(
                lambda k: wkv_t[:, KVMT * 2048 + k * DR:
                                KVMT * 2048 + (k + 1) * DR], DR,
                None, False, False, "stkpe")
            skv_bc = rsqrt_bc(zkv, KVLR, "skvbc")
            for m in range(KVMT):
                nc.vector.tensor_tensor(
                    lat_stage[:, m * TSH:(m + 1) * TSH],
                    kv_stages[m], skv_bc, op=MUL)
            # kpe rope -> packed [128,128]: token halves stacked on partitions
            sw_ps = psa.tile([DR, TSH], F32, tag="swp", bufs=2)
            nc.tensor.matmul(sw_ps, lhsT=perm_t[0:DR, 0:DR], rhs=kpe_stage,
                             start=True, stop=True)
            rt1 = pmid.tile([DR, TSH], BF, tag="rt1", bufs=2)
            nc.vector.tensor_tensor(rt1, kpe_stage, cosf_t[0:DR, :], op=MUL)
            rt2 = pmid.tile([DR, TSH], BF, tag="rt2", bufs=2)
            nc.vector.tensor_tensor(rt2, sw_ps, sinf_t[0:DR, :], op=MUL)
            half = TSH // 2
            nc.vector.tensor_tensor(
                lat_stage[0:DR, KVMT * TSH:KVMT * TSH + half],
                rt1[:, 0:half], rt2[:, 0:half], op=ADD)
            nc.vector.tensor_tensor(
                lat_stage[DR:128, KVMT * TSH:KVMT * TSH + half],
                rt1[:, half:TSH], rt2[:, half:TSH], op=ADD)
            nc.scalar.dma_start(out=contrib_lat, in_=lat_stage)  # Act: SP busy with wqb

            # q_a + norm
            zq = psa.tile([1, TSH], F32, tag="z")
            q_stages = []
            for m in range(QMT):
                q_stages.append(a_mtile(
                    lambda k, m=m: wqa_t[m // 2][:, m % 2, k * 128:(k + 1) * 128],
                    128, zq, m == 0, m == QMT - 1, f"stq{m}"))
            sq_bc = rsqrt_bc(zq, QLR, "sqbc")
            qan = []
            for m in range(QMT):
                qq = pmid.tile([128, TSH], BF, tag=f"qan{m}", name=f"qan{m}")
                nc.vector.tensor_tensor(qq, q_stages[m], sq_bc, op=MUL)
                qan.append(qq)

        # kv latent AllGather (outer scope! in-scope would gate SBUF reuse)
        nc.gpsimd.collective_compute(
            "AllGather", mybir.AluOpType.bypass,
            replica_groups=[list(range(NCORES))],
            ins=[contrib_lat], outs=[gath_lat])
        # phase-B persistent loads ride the Pool queue during the gather
        maskd_t = bcp.tile([128, 4 * 512], BF, tag="maskd", name="maskd_t")
        nc.gpsimd.dma_start(out=maskd_t, in_=maskd)
        wo_t = []
        for h in range(HPC):
            w = bcp.tile([128, HID], BF, tag=f"wo{h}", name=f"wo{h}")
            nc.gpsimd.dma_start(out=w, in_=wo[h * DV:(h + 1) * DV, :])
            wo_t.append(w)

        # ---- q_b projections for all dests ----------------------------------
        with tc.tile_pool(name="pw", bufs=1) as pw, \
             tc.tile_pool(name="psw", bufs=1, space="PSUM") as psw:
            # second half of the q_b weights
            nc.sync.dma_start(
                out=wqb_t[:, NCORES // 2:NCORES, :],
                in_=wqb[NCORES * 64:NCORES * 128, :].rearrange(
                    "(n p) km -> p n km", n=NCORES // 2))
            for d in range(NCORES):
                acc = []
                for mt in range(3):
                    acc.append(psw.tile([128, TSH], F32, tag=f"acc{mt}",
                                        bufs=2, name=f"acc{mt}"))
                for k in range(QMT):
                    for mt in range(3):
                        nc.tensor.matmul(
                            acc[mt],
                            lhsT=wqb_t[:, d, k * 384 + mt * 128:
                                       k * 384 + (mt + 1) * 128],
                            rhs=qan[k],
                            start=(k == 0), stop=(k == QMT - 1))
                for h in range(HPC):
                    nc.vector.tensor_copy(
                        q_stage[:, d * QFC + h * QHC:
                                d * QFC + h * QHC + TSH], acc[h])
                qraw = pw.tile([128, TSH], BF, tag="qraw", bufs=2)
                nc.vector.tensor_copy(qraw, acc[2])
                sw_ps = psw.tile([128, TSH], F32, tag="swp", bufs=2)
                nc.tensor.matmul(sw_ps, lhsT=perm_t, rhs=qraw,
                                 start=True, stop=True)
                rt1 = pw.tile([128, TSH], BF, tag="rt1", bufs=2)
                nc.vector.tensor_tensor(rt1, qraw, cosf_t, op=MUL)
                rt2 = pw.tile([128, TSH], BF, tag="rt2", bufs=2)
                nc.vector.tensor_tensor(rt2, sw_ps, sinf_t, op=MUL)
                half = TSH // 2
                for h in range(HPC):
                    base = d * QFC + h * QHC + TSH
                    nc.vector.tensor_tensor(
                        q_stage[0:DR, base:base + half],
                        rt1[h * DR:(h + 1) * DR, 0:half],
                        rt2[h * DR:(h + 1) * DR, 0:half], op=ADD)
                    nc.vector.tensor_tensor(
                        q_stage[DR:128, base:base + half],
                        rt1[h * DR:(h + 1) * DR, half:TSH],
                        rt2[h * DR:(h + 1) * DR, half:TSH], op=ADD)
            for h in range(HPC):
                nc.sync.dma_start(
                    out=contrib_qh[h].rearrange("(d p) f -> p d f", d=NCORES),
                    in_=q_stage.rearrange(
                        "p (d f) -> p d f", d=NCORES)[
                        :, :, h * QHC:(h + 1) * QHC])

        # ordering token: last-written q_stage region (dest 7, head 1 qpe).
        # Plain-indexed read-touches below give the post-gather load tiles a
        # WAR dependency on it, so the scheduler's collective-blind model
        # cannot hoist kv_b ahead of q_b on any engine queue.
        tok = pmid.tile([1, 16], BF, tag="tok", name="tok")
        nc.vector.tensor_copy(
            tok, q_stage[0:1, 7 * QFC + QHC + TSH:7 * QFC + QHC + TSH + 16])
        trash = pmid.tile([1, 16], BF, tag="trash", name="trash")

        # q AllToAll split per head: head-0 attention overlaps head-1 exchange
        for h in range(HPC):
            nc.gpsimd.collective_compute(
                "AllToAll", mybir.AluOpType.bypass,
                replica_groups=[list(range(NCORES))],
                ins=[contrib_qh[h]], outs=[a2a_qh[h]])

        # ---- kv_b expansion for local heads over all tokens (overlaps a2a) --
        lat = []
        kn = []
        vt = []
        gath_sv = gath_lat.rearrange("(s p) f -> p s f", s=NCORES)
        with tc.tile_pool(name="pkb", bufs=1) as pkb, \
             tc.tile_pool(name="pskb", bufs=1, space="PSUM") as pskb:
            wkvb_t = pkb.tile([128, KVMT, 4 * DN], BF, tag="wkvb")
            nc.vector.memset(wkvb_t[0:1, 0, 0:16], 0.0)
            nc.vector.tensor_tensor(trash, wkvb_t[0:1, 0, 0:16], tok, op=ADD)
            nc.scalar.dma_start(
                out=wkvb_t,
                in_=wkvb.rearrange("p (kc m) -> p kc m", kc=KVMT))
            for k in range(KVMT):
                lt = bcp.tile([128, NCORES, TSH], BF, tag=f"lat{k}",
                              name=f"lat{k}")
                nc.vector.memset(lt[0:1, 0, 0:16], 0.0)
                nc.vector.tensor_tensor(trash, lt[0:1, 0, 0:16], tok, op=ADD)
                nc.scalar.dma_start(
                    out=lt, in_=gath_sv[:, :, k * TSH:(k + 1) * TSH])
                lat.append(lt)
            kpe_all = bcp.tile([DR, NCORES, 2, TSH // 2], BF, tag="kpeall",
                               name="kpeall")
            nc.vector.memset(kpe_all[0:1, 0, 0, 0:16], 0.0)
            nc.vector.tensor_tensor(trash, kpe_all[0:1, 0, 0, 0:16], tok,
                                    op=ADD)
            nc.scalar.dma_start(
                out=kpe_all,
                in_=gath_lat.rearrange("(s two pa) f -> pa s two f",
                                       s=NCORES, two=2)[
                    :, :, :, KVMT * TSH:KVMT * TSH + TSH // 2])
            kpe_flat = kpe_all.rearrange("p s two f -> p (s two f)")
            lat_flat = [lt.rearrange("p s f -> p (s f)") for lt in lat]

            for h in range(HPC):
                knt = bcp.tile([128, T], BF, tag=f"kn{h}", name=f"kn{h}")
                for c in range(NB):
                    ps = pskb.tile([128, 512], F32, tag="knps", bufs=2)
                    for k in range(KVMT):
                        nc.tensor.matmul(
                            ps, lhsT=wkvb_t[:, k, h * DN:(h + 1) * DN],
                            rhs=lat_flat[k][:, c * 512:(c + 1) * 512],
                            start=(k == 0), stop=(k == KVMT - 1))
                    if c % 2 == 0:
                        nc.vector.tensor_copy(knt[:, c * 512:(c + 1) * 512], ps)
                    else:
                        nc.scalar.copy(knt[:, c * 512:(c + 1) * 512], ps)
                kn.append(knt)
            for tb in range(TBT):
                v = bcp.tile([128, HPC * DV], BF, tag=f"v{tb}", name=f"v{tb}")
                ps = pskb.tile([128, HPC * DV], F32, tag="vps", bufs=3)
                for k in range(KVMT):
                    nc.tensor.matmul(
                        ps, lhsT=lat_flat[k][:, tb * 128:(tb + 1) * 128],
                        rhs=wkvb_t[:, k, 2 * DN:4 * DN],
                        start=(k == 0), stop=(k == KVMT - 1))
                if tb % 2 == 0:
                    nc.vector.tensor_copy(v, ps)
                else:
                    nc.scalar.copy(v, ps)
                vt.append(v)

        # ---- Phase B: head-outer attention + per-qj output projection -------
        with tc.tile_pool(name="pc", bufs=1) as pc, \
             tc.tile_pool(name="psc", bufs=1, space="PSUM") as psc:
            qn = [[None] * NB for _ in range(HPC)]
            qpe = [[None] * NB for _ in range(HPC)]
            for h in range(HPC):
                a2a_sv = a2a_qh[h].rearrange("(s p) f -> p s f", s=NCORES)
                for qj in range(NB):
                    qn_t = pc.tile([128, 2, TSH], BF, tag=f"qn{h}_{qj}",
                                   name=f"qn{h}_{qj}")
                    nc.sync.dma_start(
                        out=qn_t,
                        in_=a2a_sv[:, 2 * qj:2 * qj + 2, 0:TSH])
                    qn[h][qj] = qn_t.rearrange("p s f -> p (s f)")
                    qpe_t = pc.tile([DR, 2, 2, TSH // 2], BF,
                                    tag=f"qpe{h}_{qj}", name=f"qpe{h}_{qj}")
                    for si, s in enumerate((2 * qj, 2 * qj + 1)):
                        nc.sync.dma_start(
                            out=qpe_t[:, si, :, :],
                            in_=a2a_qh[h][s * 128:(s + 1) * 128,
                                          TSH:QHC].rearrange(
                                "(two pa) f -> pa two f", two=2))
                    qpe[h][qj] = qpe_t.rearrange("p s two f -> p (s two f)")
            attn_n = [[None] * NB for _ in range(HPC)]

            def attend(h, qj):
                nki = 4 * qj + 4
                attn_ps = psc.tile([128, 512], F32, tag="attn", bufs=2)
                z_ps = psc.tile([1, 512], F32, tag="zr", bufs=1)
                for ki in range(nki):
                    ksl = slice(ki * 128, (ki + 1) * 128)
                    s_ps = psc.tile([128, 512], F32, tag="s", bufs=3)
                    nc.tensor.matmul(s_ps, lhsT=kn[h][:, ksl],
                                     rhs=qn[h][qj],
                                     start=True, stop=False)
                    nc.tensor.matmul(s_ps, lhsT=kpe_flat[:, ksl],
                                     rhs=qpe[h][qj],
                                     start=False, stop=True)
                    e = pc.tile([128, 512], BF, tag="e", bufs=6)
                    nc.scalar.activation(e, s_ps,
                                         mybir.ActivationFunctionType.Exp)
                    if ki >= 4 * qj:
                        sub_d = ki - 4 * qj
                        nc.vector.tensor_tensor(
                            e, e, maskd_t[:, sub_d * 512:(sub_d + 1) * 512],
                            op=MUL)
                    nc.tensor.matmul(z_ps, lhsT=ones_col, rhs=e,
                                     start=(ki == 0), stop=(ki == nki - 1))
                    nc.tensor.matmul(attn_ps,
                                     lhsT=vt[ki][:, h * DV:(h + 1) * DV],
                                     rhs=e,
                                     start=(ki == 0), stop=(ki == nki - 1))
                rz = pc.tile([1, 512], BF, tag="rz", bufs=2)
                with nc.allow_low_precision(reason="bf16 softmax denom"):
                    nc.vector.reciprocal(rz, z_ps)
                bc_ps = psc.tile([128, 512], F32, tag="s", bufs=3)
                nc.tensor.matmul(bc_ps, lhsT=ones_row, rhs=rz,
                                 start=True, stop=True)
                bc_sb = pc.tile([128, 512], BF, tag="bcs", bufs=2)
                nc.scalar.copy(bc_sb, bc_ps)
                attn_n[h][qj] = pc.tile([128, 512], BF, tag=f"at{h}_{qj}",
                                        name=f"attnn{h}_{qj}")
                nc.vector.tensor_tensor(attn_n[h][qj], attn_ps, bc_sb, op=MUL)

            def outproj(qj):
                for tt in range(4):
                    tb = qj * 4 + tt
                    tsl = slice(tt * 128, (tt + 1) * 128)
                    o_row = pc.tile([128, HID], BF, tag="orow", bufs=3)
                    for hb in range(NB):
                        o_ps = psc.tile([128, 512], F32, tag="o", bufs=2)
                        for h in range(HPC):
                            nc.tensor.matmul(
                                o_ps,
                                lhsT=attn_n[h][qj][:, tsl],
                                rhs=wo_t[h][:, hb * 512:(hb + 1) * 512],
                                start=(h == 0),
                                stop=(h == HPC - 1),
                            )
                        if hb % 2 == 0:
                            nc.vector.tensor_copy(
                                o_row[:, hb * 512:(hb + 1) * 512], o_ps)
                        else:
                            nc.scalar.copy(
                                o_row[:, hb * 512:(hb + 1) * 512], o_ps)
                    if tt % 2 == 0:
                        nc.scalar.dma_start(
                            out=out[tb * 128:(tb + 1) * 128, :], in_=o_row)
                    else:
                        nc.sync.dma_start(
                            out=out[tb * 128:(tb + 1) * 128, :], in_=o_row)

            for qj in range(NB):
                attend(0, qj)
            for qj in range(NB):
                attend(1, qj)
                outproj(qj)


_NC_CACHE = {}


def _get_nc():
    if "nc" not in _NC_CACHE:
        _NC_CACHE["nc"] = build_bass()
    return _NC_CACHE["nc"]


def make_in_maps(positions, hidden_states, w_q_a, q_a_ln_w, w_q_b, w_kv_a,
                 kv_a_ln_w, w_kv_b, w_o):
    BF_NP = mybir.dt.np(mybir.dt.bfloat16)

    positions = np.asarray(positions)
    hidden_states = np.asarray(hidden_states, dtype=np.float32)
    w_q_a = np.asarray(w_q_a, dtype=np.float32)
    q_a_ln_w = np.asarray(q_a_ln_w, dtype=np.float32)
    w_q_b = np.asarray(w_q_b, dtype=np.float32)
    w_kv_a = np.asarray(w_kv_a, dtype=np.float32)
    kv_a_ln_w = np.asarray(kv_a_ln_w, dtype=np.float32)
    w_kv_b = np.asarray(w_kv_b, dtype=np.float32)
    w_o = np.asarray(w_o, dtype=np.float32)

    hs_t = np.ascontiguousarray(hidden_states.T)

    # deinterleave rope features: evens then odds (dot-products invariant)
    order = np.concatenate([np.arange(0, DR, 2), np.arange(1, DR, 2)])

    wkva_p = w_kv_a.copy()
    wkva_p[:, KVLR:] = w_kv_a[:, KVLR:][:, order]

    inv_freq = 1.0 / (THETA ** (np.arange(0, DR, 2, dtype=np.float64) / DR))
    ang = positions.astype(np.float64)[:, None] * inv_freq[None, :]
    cosT = np.cos(ang).T
    sinT = np.sin(ang).T
    cosf = np.concatenate([cosT, cosT], axis=0)        # [64, T]
    sinf = np.concatenate([-sinT, sinT], axis=0)       # [64, T]
    cosf2 = np.concatenate([cosf, cosf], axis=0)       # [128, T] dual-head
    sinf2 = np.concatenate([sinf, sinf], axis=0)

    perm64 = np.zeros((DR, DR), dtype=np.float32)
    for i in range(DR):
        perm64[i, (i + DR // 2) % DR] = 1.0
    perm128 = np.zeros((128, 128), dtype=np.float32)
    perm128[:DR, :DR] = perm64
    perm128[DR:, DR:] = perm64

    maskd = np.zeros((128, 4 * 512), dtype=np.float32)
    p = np.arange(128)[:, None]
    f = np.arange(512)[None, :]
    for sub in range(4):
        maskd[:, sub * 512:(sub + 1) * 512] = (p + 128 * sub <= f)

    # all-heads q_b weights, columns grouped per destination core:
    # [qn h0 (128) | qn h1 (128) | qpe h0 perm (64) | qpe h1 perm (64)]
    wqb_all = np.concatenate([
        np.concatenate([
            w_q_b[:, h0 * DQK:h0 * DQK + DN],
            w_q_b[:, h1 * DQK:h1 * DQK + DN],
            w_q_b[:, h0 * DQK + DN:(h0 + 1) * DQK][:, order],
            w_q_b[:, h1 * DQK + DN:(h1 + 1) * DQK][:, order],
        ], axis=1)
        for h0, h1 in ((2 * d, 2 * d + 1) for d in range(NCORES))
    ], axis=1) * q_a_ln_w[:, None] * SCALE

    def pack(w, mrows):
        # [K, M] -> strip-major [nstrips*128, (K/128)*mrows]: each strip row-
        # contiguous so the device DMA is 128 fat descriptors
        Kd, Md = w.shape
        n = Md // mrows
        return np.ascontiguousarray(
            w.reshape(Kd // 128, 128, n, mrows).transpose(2, 1, 0, 3)
            .reshape(n * 128, (Kd // 128) * mrows)).astype(BF_NP)

    wqa_pk = pack(w_q_a, 128)
    wkva_pk = pack(wkva_p[:, :KVLR], 128)         # [4*128, 16*128]
    wkpe_pk = pack(wkva_p[:, KVLR:], DR)          # [128, 16*64]
    # single [128, *] row: kv strips side by side, then the kpe strip
    wkv_all = np.concatenate([
        wkva_pk.reshape(KVMT, 128, KT * 128).transpose(1, 0, 2).reshape(
            128, KVMT * KT * 128),
        wkpe_pk,
    ], axis=1).astype(BF_NP)
    wqb_pk = pack(wqb_all, HPC * DQK)

    in_maps = []
    for c in range(NCORES):
        h0, h1 = HPC * c, HPC * c + 1
        # per-core kv_b: cols [kn h0 | kn h1 | v h0 | v h1], ln folded
        wkvb_c = np.concatenate([
            w_kv_b[:, h0 * (DN + DV):h0 * (DN + DV) + DN],
            w_kv_b[:, h1 * (DN + DV):h1 * (DN + DV) + DN],
            w_kv_b[:, h0 * (DN + DV) + DN:(h0 + 1) * (DN + DV)],
            w_kv_b[:, h1 * (DN + DV) + DN:(h1 + 1) * (DN + DV)],
        ], axis=1) * kv_a_ln_w[:, None]
        wkvb_pk = pack(wkvb_c, 4 * DN)
        wo_c = np.concatenate([
            w_o[h0 * DV:(h0 + 1) * DV, :],
            w_o[h1 * DV:(h1 + 1) * DV, :],
        ], axis=0)
        tsl = slice(c * TSH, (c + 1) * TSH)
        in_maps.append({
            "hs_sh": np.ascontiguousarray(hs_t[:, tsl]).astype(BF_NP),
            "wqa": wqa_pk,
            "wkv_all": wkv_all,
            "wqb": wqb_pk,
            "wkvb": wkvb_pk,
            "wo": np.ascontiguousarray(wo_c).astype(BF_NP),
            "cosf_sh": np.ascontiguousarray(cosf2[:, tsl]).astype(BF_NP),
            "sinf_sh": np.ascontiguousarray(sinf2[:, tsl]).astype(BF_NP),
            "perm128": perm128.astype(BF_NP),
            "maskd": np.ascontiguousarray(maskd).astype(BF_NP),
            "ones": np.ones((128, 128), dtype=np.float32).astype(BF_NP),
        })
    return in_maps


def kernel(positions, hidden_states, w_q_a, q_a_ln_w, w_q_b, w_kv_a,
           kv_a_ln_w, w_kv_b, w_o):
    nc = _get_nc()
    in_maps = make_in_maps(positions, hidden_states, w_q_a, q_a_ln_w, w_q_b,
                           w_kv_a, kv_a_ln_w, w_kv_b, w_o)
    res = bass_utils.run_bass_kernel_spmd(nc, in_maps, core_ids=list(range(NCORES)))
    acc = np.zeros((T, HID), dtype=np.float32)
    for c in range(NCORES):
        acc += np.asarray(res.results[c]["out"], dtype=np.float32)
    return acc
